# revision 1
# baseline (speedup 1.0000x reference)
"""Trainium2 Bass kernel for nn_LocalEncoder (masked GRU + attention pooling).

Strategy (data-parallel over batch, 8 cores x 512 rows, 2 chunks of 256/core):
- Feature-major layout [U partitions, batch free]. All matmuls bf16 -> fp32 PSUM.
- Scan: per timestep, 6 matmuls (x-proj + recurrent) with biases and the
  trailing-padding mask folded in via augmented x rows (mask row scaled by -40
  makes the update gate ~0 on masked steps, freezing h exactly like the
  reference's jnp.where).
- all_state spilled to DRAM (bf16), re-streamed for the attention phase.
- Attention computed with the last-state term UNMASKED on device; the host
  subtracts the closed-form correction for masked (trailing) timesteps:
  masked t contribute (T-len)*sigmoid(last@A2)@v * last, device counted
  (T-len)*sigmoid(last@A2 + last@A1)@v * last.
"""
import sys
sys.path.insert(0, "/opt/trn_rl_repo")
from contextlib import ExitStack

import numpy as np
import ml_dtypes

import concourse.bass as bass
import concourse.bacc as bacc
import concourse.tile as tile
from concourse import mybir
from concourse import bass_utils

bf16 = ml_dtypes.bfloat16
AF = mybir.ActivationFunctionType
OP = mybir.AluOpType

B, T, E, U = 4096, 200, 100, 100
NCORES = 8
BC = 256          # chunk width (free dim of every op)
NCHUNK = 2        # chunks per core; BC*NCHUNK = per-core batch
PERCORE = BC * NCHUNK

_CACHE = {}


def _build():
    nc = bacc.Bacc()
    dt = mybir.dt
    xaug = nc.dram_tensor("xaug", [T, NCHUNK, 128, BC], dt.bfloat16, kind="ExternalInput")
    wKzN = nc.dram_tensor("wKzN", [128, U], dt.bfloat16, kind="ExternalInput")
    wKr = nc.dram_tensor("wKr", [128, U], dt.bfloat16, kind="ExternalInput")
    wKh = nc.dram_tensor("wKh", [128, U], dt.bfloat16, kind="ExternalInput")
    wRzN = nc.dram_tensor("wRzN", [U, U], dt.bfloat16, kind="ExternalInput")
    wRr = nc.dram_tensor("wRr", [U, U], dt.bfloat16, kind="ExternalInput")
    wRh = nc.dram_tensor("wRh", [U, U], dt.bfloat16, kind="ExternalInput")
    wb1h = nc.dram_tensor("wb1h", [1, U], dt.bfloat16, kind="ExternalInput")
    wA1 = nc.dram_tensor("wA1", [U, U], dt.bfloat16, kind="ExternalInput")
    wA2 = nc.dram_tensor("wA2", [U, U], dt.bfloat16, kind="ExternalInput")
    wVr = nc.dram_tensor("wVr", [U, U], dt.bfloat16, kind="ExternalInput")
    wI = nc.dram_tensor("wI", [U, U], dt.bfloat16, kind="ExternalInput")
    outraw = nc.dram_tensor("outraw", [NCHUNK, U, BC], dt.float32, kind="ExternalOutput")
    lastout = nc.dram_tensor("lastout", [NCHUNK, U, BC], dt.float32, kind="ExternalOutput")

    with tile.TileContext(nc) as tc, ExitStack() as octx:
        singles = octx.enter_context(tc.tile_pool(name="singles", bufs=1))
        dram = octx.enter_context(tc.tile_pool(name="dram", bufs=1, space="DRAM"))

        # persistent weights
        def load_w(dram_w, p):
            t = singles.tile([p, U], mybir.dt.bfloat16, tag=dram_w.name)
            nc.sync.dma_start(out=t, in_=dram_w[:, :])
            return t
        KzN, Kr, Kh = load_w(wKzN, 128), load_w(wKr, 128), load_w(wKh, 128)
        RzN, Rr, Rh = load_w(wRzN, U), load_w(wRr, U), load_w(wRh, U)
        A1b, A2b, Vr, I100 = load_w(wA1, U), load_w(wA2, U), load_w(wVr, U), load_w(wI, U)
        b1h = singles.tile([1, U], mybir.dt.bfloat16, tag="b1h")
        nc.sync.dma_start(out=b1h, in_=wb1h[:, :])
        ones = singles.tile([1, BC], mybir.dt.bfloat16, tag="ones")
        nc.vector.memset(ones, 1.0)

        state = dram.tile([NCHUNK, T, U, BC], mybir.dt.bfloat16)
        last_tiles = []

        # ---------------- scan ----------------
        with ExitStack() as ctx:
            xp = ctx.enter_context(tc.tile_pool(name="xp", bufs=4))
            hp = ctx.enter_context(tc.tile_pool(name="hp", bufs=3))
            gp = ctx.enter_context(tc.tile_pool(name="gp", bufs=3))
            pzr = ctx.enter_context(tc.tile_pool(name="pzr", bufs=1, space="PSUM"))
            pxh = ctx.enter_context(tc.tile_pool(name="pxh", bufs=1, space="PSUM"))
            prh = ctx.enter_context(tc.tile_pool(name="prh", bufs=1, space="PSUM"))

            hprev = [None] * NCHUNK
            for c in range(NCHUNK):
                h0 = hp.tile([128, BC], mybir.dt.bfloat16, tag=f"h{c}")
                nc.vector.memset(h0, 0.0)
                hprev[c] = h0

            for t in range(T):
                for c in range(NCHUNK):
                    xt = xp.tile([128, BC], mybir.dt.bfloat16, tag=f"x{c}")
                    nc.sync.dma_start(out=xt, in_=xaug[t, c, :, :])
                    h = hprev[c]
                    zr = pzr.tile([128, 2, 512], mybir.dt.float32, tag=f"zr{c}")
                    xh = pxh.tile([128, 512], mybir.dt.float32, tag=f"xh{c}")
                    rh = prh.tile([128, 512], mybir.dt.float32, tag=f"rh{c}")
                    nc.tensor.matmul(zr[0:U, 0, 0:BC], lhsT=KzN, rhs=xt, start=True, stop=False)
                    nc.tensor.matmul(zr[0:U, 0, 0:BC], lhsT=RzN, rhs=h[0:U, :], start=False, stop=True)
                    nc.tensor.matmul(zr[0:U, 1, 0:BC], lhsT=Kr, rhs=xt, start=True, stop=False)
                    nc.tensor.matmul(zr[0:U, 1, 0:BC], lhsT=Rr, rhs=h[0:U, :], start=False, stop=True)
                    nc.tensor.matmul(xh[0:U, 0:BC], lhsT=Kh, rhs=xt, start=True, stop=True)
                    nc.tensor.matmul(rh[0:U, 0:BC], lhsT=Rh, rhs=h[0:U, :], start=True, stop=False)
                    nc.tensor.matmul(rh[0:U, 0:BC], lhsT=b1h, rhs=ones, start=False, stop=True)
                    # gates: one sigmoid over both banks (zcm | r)
                    zrs = gp.tile([U, 2, BC], mybir.dt.bfloat16, tag=f"zrs{c}")
                    nc.scalar.activation(zrs[:, :, :], zr[0:U, :, 0:BC], AF.Sigmoid)
                    t1 = gp.tile([U, BC], mybir.dt.bfloat16, tag=f"t1{c}")
                    nc.vector.tensor_tensor(t1, zrs[:, 1, :], rh[0:U, 0:BC], OP.mult)
                    s = gp.tile([U, BC], mybir.dt.bfloat16, tag=f"s{c}")
                    nc.vector.tensor_tensor(s, xh[0:U, 0:BC], t1, OP.add)
                    hh = gp.tile([U, BC], mybir.dt.bfloat16, tag=f"hh{c}")
                    nc.scalar.activation(hh, s, AF.Tanh)
                    d = gp.tile([U, BC], mybir.dt.bfloat16, tag=f"d{c}")
                    nc.vector.tensor_tensor(d, hh, h[0:U, :], OP.subtract)
                    e = gp.tile([U, BC], mybir.dt.bfloat16, tag=f"e{c}")
                    nc.vector.tensor_tensor(e, zrs[:, 0, :], d, OP.mult)
                    hn = hp.tile([128, BC], mybir.dt.bfloat16, tag=f"h{c}")
                    nc.vector.tensor_tensor(hn[0:U, :], h[0:U, :], e, OP.add)
                    nc.sync.dma_start(out=state[c, t, :, :], in_=hn[0:U, :])
                    hprev[c] = hn

            for c in range(NCHUNK):
                lt = singles.tile([128, BC], mybir.dt.bfloat16, tag=f"last{c}")
                nc.vector.tensor_copy(lt[0:U, :], hprev[c][0:U, :])
                last_tiles.append(lt)
                lo = singles.tile([U, BC], mybir.dt.float32, tag=f"lasto{c}")
                nc.vector.tensor_copy(lo, hprev[c][0:U, :])
                nc.sync.dma_start(out=lastout[c, :, :], in_=lo)

        # ---------------- attention ----------------
        with ExitStack() as ctx:
            sp = ctx.enter_context(tc.tile_pool(name="sp", bufs=4))
            gp2 = ctx.enter_context(tc.tile_pool(name="gp2", bufs=3))
            ps = ctx.enter_context(tc.tile_pool(name="ps", bufs=1, space="PSUM"))
            pa = ctx.enter_context(tc.tile_pool(name="pa", bufs=1, space="PSUM"))
            po = ctx.enter_context(tc.tile_pool(name="po", bufs=1, space="PSUM"))

            for c in range(NCHUNK):
                acc = po.tile([128, 512], mybir.dt.float32, tag=f"acc{c}")
                for t in range(T):
                    st = sp.tile([U, BC], mybir.dt.bfloat16, tag=f"st{c}")
                    nc.sync.dma_start(out=st, in_=state[c, t, :, :])
                    sb = ps.tile([128, 512], mybir.dt.float32, tag=f"sb{c}")
                    nc.tensor.matmul(sb[0:U, 0:BC], lhsT=A2b, rhs=st, start=True, stop=False)
                    nc.tensor.matmul(sb[0:U, 0:BC], lhsT=A1b, rhs=last_tiles[c][0:U, :], start=False, stop=True)
                    g = gp2.tile([U, BC], mybir.dt.bfloat16, tag=f"g{c}")
                    nc.scalar.activation(g, sb[0:U, 0:BC], AF.Sigmoid)
                    al = pa.tile([128, 512], mybir.dt.float32, tag=f"al{c}")
                    nc.tensor.matmul(al[0:U, 0:BC], lhsT=Vr, rhs=g, start=True, stop=True)
                    tmp = gp2.tile([U, BC], mybir.dt.bfloat16, tag=f"tmp{c}")
                    nc.vector.tensor_tensor(tmp, al[0:U, 0:BC], st, OP.mult)
                    nc.tensor.matmul(acc[0:U, 0:BC], lhsT=I100, rhs=tmp,
                                     start=(t == 0), stop=(t == T - 1))
                osb = gp2.tile([U, BC], mybir.dt.float32, tag=f"osb{c}")
                nc.vector.tensor_copy(osb, acc[0:U, 0:BC])
                nc.sync.dma_start(out=outraw[c, :, :], in_=osb)

    nc.compile()
    return nc


def _prep_weights(kernel_w, rec_kernel, bias):
    b0, b1 = bias[0], bias[1]
    w = {}
    KzN = np.zeros((128, U), np.float32)
    KzN[:E] = -kernel_w[:, :U]
    KzN[100, :] = -40.0
    KzN[101, :] = -(b0[:U] + b1[:U])
    Kr = np.zeros((128, U), np.float32)
    Kr[:E] = kernel_w[:, U:2 * U]
    Kr[101, :] = b0[U:2 * U] + b1[U:2 * U]
    Kh = np.zeros((128, U), np.float32)
    Kh[:E] = kernel_w[:, 2 * U:]
    Kh[101, :] = b0[2 * U:]
    w["wKzN"], w["wKr"], w["wKh"] = KzN, Kr, Kh
    w["wRzN"] = -rec_kernel[:, :U]
    w["wRr"] = rec_kernel[:, U:2 * U]
    w["wRh"] = rec_kernel[:, 2 * U:]
    w["wb1h"] = b1[2 * U:][None, :]
    return {k: v.astype(bf16) for k, v in w.items()}


def kernel(session_hidden, mask, kernel, rec_kernel, bias, A1_w, A2_w, v):
    session_hidden = np.asarray(session_hidden, np.float32)
    mask = np.asarray(mask, np.float32)
    kernel_w = np.asarray(kernel, np.float32)
    rec_kernel = np.asarray(rec_kernel, np.float32)
    bias = np.asarray(bias, np.float32)
    A1_w = np.asarray(A1_w, np.float32)
    A2_w = np.asarray(A2_w, np.float32)
    v = np.asarray(v, np.float32)

    if "nc" not in _CACHE:
        _CACHE["nc"] = _build()
    nc = _CACHE["nc"]

    w = _prep_weights(kernel_w, rec_kernel, bias)
    w["wA1"] = A1_w.astype(bf16)
    w["wA2"] = A2_w.astype(bf16)
    w["wVr"] = np.broadcast_to(v[0][:, None], (U, U)).astype(bf16).copy()
    w["wI"] = np.eye(U, dtype=np.float32).astype(bf16)

    # xaug: [T, NCHUNK, 128, BC] per core; rows 0:100 = x^T, 100 = 1-m, 101 = 1
    x = session_hidden.reshape(NCORES, NCHUNK, BC, T, E)
    m = mask.reshape(NCORES, NCHUNK, BC, T)
    in_maps = []
    for k in range(NCORES):
        xa = np.zeros((T, NCHUNK, 128, BC), np.float32)
        xa[:, :, :E, :] = x[k].transpose(2, 0, 3, 1)   # [T, chunk, E, BC]
        xa[:, :, 100, :] = 1.0 - m[k].transpose(2, 0, 1)
        xa[:, :, 101, :] = 1.0
        im = dict(w)
        im["xaug"] = xa.astype(bf16)
        in_maps.append(im)

    res = bass_utils.run_bass_kernel_spmd(nc, in_maps, core_ids=list(range(NCORES)))

    out_raw = np.zeros((B, U), np.float32)
    last = np.zeros((B, U), np.float32)
    for k in range(NCORES):
        r = res.results[k]
        for c in range(NCHUNK):
            sl = slice(k * PERCORE + c * BC, k * PERCORE + (c + 1) * BC)
            out_raw[sl] = np.asarray(r["outraw"][c]).T.astype(np.float32)
            last[sl] = np.asarray(r["lastout"][c]).T.astype(np.float32)

    # host correction for masked timesteps (device used last@A1 term for ALL t)
    lengths = mask.sum(1)
    sl_ = last @ A2_w
    c_ = last @ A1_w
    sig = lambda a: 1.0 / (1.0 + np.exp(-a))
    a_corr = (sig(sl_ + c_) - sig(sl_)) @ v[0]
    out = out_raw - (T - lengths)[:, None] * a_corr[:, None] * last
    return out.astype(np.float32)



# revision 2
# speedup vs baseline: 1.0129x; 1.0129x over previous
"""Trainium2 Bass kernel for nn_LocalEncoder (masked GRU + attention pooling), v2.

Data-parallel 8 cores x 512 rows (2 chunks x 256). Feature-major [U, batch].
Scan: 6 bf16 matmuls/chunk-step (biases + mask folded into augmented weights,
outputs 101 partitions wide so the constant-1 row regenerates itself), sigmoid
on ACT, r*rh and xh+t1 on GpSimd, tanh on ACT, (hh-h), z'*(.), h+(.) on DVE in
4x scalar_tensor_tensor mode. State spilled to DRAM bf16 (x-in on qSP HWDGE,
state-out on qAct HWDGE). Attention: W=4 timestep groups, A2/Vr matmuls,
sigmoid batched, al*st on GpSimd, accumulate on DVE; host corrects masked steps
(same closed form as before).
"""
import sys
sys.path.insert(0, "/opt/trn_rl_repo")
from contextlib import ExitStack

import numpy as np
import ml_dtypes

import concourse.bass as bass
import concourse.bacc as bacc
import concourse.tile as tile
from concourse import mybir
from concourse import bass_utils

bf16 = ml_dtypes.bfloat16
AF = mybir.ActivationFunctionType
OP = mybir.AluOpType

B, T, E, U = 4096, 200, 100, 100
NCORES = 8
BC = 256
NCHUNK = 2
PERCORE = BC * NCHUNK
WX = 8            # x-load DMA batch (timesteps)
WA = 4            # attention sigmoid batch (timesteps)
UP = U + 1        # augmented output width (row 100 = const 1)

_CACHE = {}


def _build():
    nc = bacc.Bacc()
    dt = mybir.dt
    # xaug: [NCHUNK, T/WX, 128, WX, BC] so one DMA grabs WX steps contiguously
    xaug = nc.dram_tensor("xaug", [NCHUNK, T // WX, 128, WX, BC], dt.bfloat16,
                          kind="ExternalInput")
    wKz = nc.dram_tensor("wKz", [128, UP], dt.bfloat16, kind="ExternalInput")
    wKr = nc.dram_tensor("wKr", [128, UP], dt.bfloat16, kind="ExternalInput")
    wKh = nc.dram_tensor("wKh", [128, UP], dt.bfloat16, kind="ExternalInput")
    wRz = nc.dram_tensor("wRz", [UP, UP], dt.bfloat16, kind="ExternalInput")
    wRr = nc.dram_tensor("wRr", [UP, UP], dt.bfloat16, kind="ExternalInput")
    wRh = nc.dram_tensor("wRh", [UP, UP], dt.bfloat16, kind="ExternalInput")
    wA1 = nc.dram_tensor("wA1", [U, U], dt.bfloat16, kind="ExternalInput")
    wA2 = nc.dram_tensor("wA2", [U, U], dt.bfloat16, kind="ExternalInput")
    wVr = nc.dram_tensor("wVr", [U, U], dt.bfloat16, kind="ExternalInput")
    outraw = nc.dram_tensor("outraw", [NCHUNK, U, BC], dt.float32, kind="ExternalOutput")
    lastout = nc.dram_tensor("lastout", [NCHUNK, U, BC], dt.float32, kind="ExternalOutput")

    with tile.TileContext(nc) as tc, ExitStack() as octx:
        singles = octx.enter_context(tc.tile_pool(name="singles", bufs=1))
        dram = octx.enter_context(tc.tile_pool(name="dram", bufs=1, space="DRAM"))

        def load_w(dram_w, p, m):
            t = singles.tile([p, m], mybir.dt.bfloat16, tag=dram_w.name)
            nc.sync.dma_start(out=t, in_=dram_w[:, :])
            return t
        Kz, Kr, Kh = load_w(wKz, 128, UP), load_w(wKr, 128, UP), load_w(wKh, 128, UP)
        Rz, Rr, Rh = load_w(wRz, UP, UP), load_w(wRr, UP, UP), load_w(wRh, UP, UP)
        A1b, A2b, Vr = load_w(wA1, U, U), load_w(wA2, U, U), load_w(wVr, U, U)

        state = dram.tile([NCHUNK, U, T, BC], mybir.dt.bfloat16)
        last_tiles = []

        # ---------------- scan ----------------
        with ExitStack() as ctx:
            xp = ctx.enter_context(tc.tile_pool(name="xp", bufs=3))
            hp = ctx.enter_context(tc.tile_pool(name="hp", bufs=4))
            gp = ctx.enter_context(tc.tile_pool(name="gp", bufs=3))
            pzr = ctx.enter_context(tc.tile_pool(name="pzr", bufs=1, space="PSUM"))
            phc = ctx.enter_context(tc.tile_pool(name="phc", bufs=1, space="PSUM"))

            hprev = [None] * NCHUNK
            for c in range(NCHUNK):
                h0 = hp.tile([128, BC], mybir.dt.bfloat16, tag=f"h{c}")
                nc.vector.memset(h0, 0.0)
                nc.vector.memset(h0[96:128, :], 1.0)
                nc.vector.memset(h0[96:100, :], 0.0)
                hprev[c] = h0

            xts = [None] * NCHUNK
            for t in range(T):
                ib = t % WX
                if ib == 0:
                    for c in range(NCHUNK):
                        xt = xp.tile([128, WX, BC], mybir.dt.bfloat16, tag=f"x{c}")
                        nc.sync.dma_start(out=xt, in_=xaug[c, t // WX, :, :, :])
                        xts[c] = xt
                zr = [None] * NCHUNK
                hc = [None] * NCHUNK
                for c in range(NCHUNK):
                    zr[c] = pzr.tile([128, 2, BC], mybir.dt.float32, tag=f"zr{c}",
                                     name=f"zr{c}")
                    hc[c] = phc.tile([128, 2, BC], mybir.dt.float32, tag=f"hc{c}",
                                     name=f"hc{c}")
                # weight-paired emission across chunks
                for c in range(NCHUNK):
                    nc.tensor.matmul(zr[c][0:UP, 0, :], lhsT=Kz, rhs=xts[c][:, ib, :],
                                     start=True, stop=False, skip_group_check=True)
                for c in range(NCHUNK):
                    nc.tensor.matmul(zr[c][0:UP, 0, :], lhsT=Rz, rhs=hprev[c][0:UP, :],
                                     start=False, stop=True, skip_group_check=True)
                for c in range(NCHUNK):
                    nc.tensor.matmul(zr[c][0:UP, 1, :], lhsT=Kr, rhs=xts[c][:, ib, :],
                                     start=True, stop=False, skip_group_check=True)
                for c in range(NCHUNK):
                    nc.tensor.matmul(zr[c][0:UP, 1, :], lhsT=Rr, rhs=hprev[c][0:UP, :],
                                     start=False, stop=True, skip_group_check=True)
                for c in range(NCHUNK):
                    nc.tensor.matmul(hc[c][0:UP, 0, :], lhsT=Kh, rhs=xts[c][:, ib, :],
                                     start=True, stop=True, skip_group_check=True)
                for c in range(NCHUNK):
                    nc.tensor.matmul(hc[c][0:UP, 1, :], lhsT=Rh, rhs=hprev[c][0:UP, :],
                                     start=True, stop=True, skip_group_check=True)
                for c in range(NCHUNK):
                    h = hprev[c]
                    zrs = gp.tile([UP, 2, BC], mybir.dt.bfloat16, tag=f"zrs{c}")
                    nc.scalar.activation(zrs, zr[c][0:UP, :, :], AF.Sigmoid)
                    t1 = gp.tile([UP, BC], mybir.dt.bfloat16, tag=f"t1{c}")
                    nc.vector.tensor_tensor(t1, zrs[:, 1, :], hc[c][0:UP, 1, :], OP.mult)
                    s = gp.tile([UP, BC], mybir.dt.bfloat16, tag=f"s{c}")
                    nc.vector.tensor_tensor(s, hc[c][0:UP, 0, :], t1, OP.add)
                    hh = gp.tile([UP, BC], mybir.dt.bfloat16, tag=f"hh{c}")
                    nc.scalar.activation(hh, s, AF.Tanh)
                    d = gp.tile([UP, BC], mybir.dt.bfloat16, tag=f"d{c}")
                    nc.vector.tensor_tensor(d, hh, h[0:UP, :], OP.subtract)
                    e = gp.tile([UP, BC], mybir.dt.bfloat16, tag=f"e{c}")
                    nc.vector.tensor_tensor(e, zrs[:, 0, :], d, OP.mult)
                    hn = hp.tile([128, BC], mybir.dt.bfloat16, tag=f"h{c}")
                    nc.vector.tensor_tensor(hn[0:UP, :], e, h[0:UP, :], OP.add)
                    nc.scalar.dma_start(out=state[c, :, t, :], in_=hn[0:U, :])
                    hprev[c] = hn

            for c in range(NCHUNK):
                lt = singles.tile([128, BC], mybir.dt.bfloat16, tag=f"last{c}")
                nc.vector.tensor_copy(lt[0:UP, :], hprev[c][0:UP, :])
                last_tiles.append(lt)
                lo = singles.tile([U, BC], mybir.dt.float32, tag=f"lasto{c}")
                nc.vector.tensor_copy(lo, hprev[c][0:U, :])
                nc.scalar.dma_start(out=lastout[c, :, :], in_=lo)

        # ---------------- attention ----------------
        NG = T // WA
        with ExitStack() as ctx:
            sp = ctx.enter_context(tc.tile_pool(name="sp", bufs=3))
            gp2 = ctx.enter_context(tc.tile_pool(name="gp2", bufs=3))
            ap2 = ctx.enter_context(tc.tile_pool(name="ap2", bufs=1))
            psb = ctx.enter_context(tc.tile_pool(name="psb", bufs=1, space="PSUM"))
            pal = ctx.enter_context(tc.tile_pool(name="pal", bufs=1, space="PSUM"))

            c4s, accs = [], []
            for c in range(NCHUNK):
                # c4 = A1 @ last, duplicated over the WA slots
                sb1 = psb.tile([128, WA, BC], mybir.dt.float32, tag=f"sb{c}")
                nc.tensor.matmul(sb1[0:U, 0, :], lhsT=A1b, rhs=last_tiles[c][0:U, :],
                                 start=True, stop=True)
                c4 = ap2.tile([U, WA, BC], mybir.dt.bfloat16, tag=f"c4{c}")
                nc.vector.tensor_copy(c4[:, 0, :], sb1[0:U, 0, :])
                nc.gpsimd.tensor_copy(c4[:, 1, :], c4[:, 0, :])
                nc.gpsimd.tensor_copy(c4[:, 2:4, :], c4[:, 0:2, :])
                c4s.append(c4)
                acc = ap2.tile([U, WA, BC], mybir.dt.float32, tag=f"acc{c}")
                nc.vector.memset(acc, 0.0)
                accs.append(acc)

            for g in range(NG):
                for c in range(NCHUNK):
                    st4 = sp.tile([U, WA, BC], mybir.dt.bfloat16, tag=f"st{c}")
                    nc.sync.dma_start(out=st4, in_=state[c, :, g * WA:(g + 1) * WA, :])
                    sb4 = psb.tile([128, WA, BC], mybir.dt.float32, tag=f"sb{c}")
                    nc.tensor.matmul(sb4[0:U, 0:2, :], lhsT=A2b, rhs=st4[:, 0:2, :],
                                     start=True, stop=True)
                    nc.tensor.matmul(sb4[0:U, 2:4, :], lhsT=A2b, rhs=st4[:, 2:4, :],
                                     start=True, stop=True)
                    sbc = gp2.tile([U, WA, BC], mybir.dt.bfloat16, tag=f"sbc{c}")
                    nc.vector.tensor_tensor(sbc, sb4[0:U, :, :], c4s[c], OP.add)
                    g4 = gp2.tile([U, WA, BC], mybir.dt.bfloat16, tag=f"g4{c}")
                    nc.scalar.activation(g4, sbc, AF.Sigmoid)
                    al4 = pal.tile([128, WA, BC], mybir.dt.float32, tag=f"al{c}")
                    nc.tensor.matmul(al4[0:U, 0:2, :], lhsT=Vr, rhs=g4[:, 0:2, :],
                                     start=True, stop=True)
                    nc.tensor.matmul(al4[0:U, 2:4, :], lhsT=Vr, rhs=g4[:, 2:4, :],
                                     start=True, stop=True)
                    tmp = gp2.tile([U, WA, BC], mybir.dt.bfloat16, tag=f"tmp{c}")
                    nc.vector.tensor_tensor(tmp, al4[0:U, :, :], st4, OP.mult)
                    nc.gpsimd.tensor_tensor(accs[c], accs[c], tmp, OP.add)

            for c in range(NCHUNK):
                osb = gp2.tile([U, BC], mybir.dt.float32, tag=f"osb{c}")
                nc.vector.tensor_reduce(
                    osb, accs[c].rearrange("u w b -> u b w"), mybir.AxisListType.X,
                    OP.add)
                nc.scalar.dma_start(out=outraw[c, :, :], in_=osb)

    nc.compile()
    return nc


def _prep_weights(kernel_w, rec_kernel, bias, A1_w, A2_w, v):
    b0, b1 = bias[0], bias[1]
    w = {}
    Kz = np.zeros((128, UP), np.float32)
    Kz[:E, :U] = -kernel_w[:, :U]
    Kz[100, :U] = -40.0
    Kz[101, :U] = -(b0[:U] + b1[:U])
    Kz[101, 100] = -40.0          # keeps row 100 of h at sigma(-40)*... = frozen 1
    Kr = np.zeros((128, UP), np.float32)
    Kr[:E, :U] = kernel_w[:, U:2 * U]
    Kr[101, :U] = b0[U:2 * U] + b1[U:2 * U]
    Kh = np.zeros((128, UP), np.float32)
    Kh[:E, :U] = kernel_w[:, 2 * U:]
    Kh[101, :U] = b0[2 * U:]
    Rz = np.zeros((UP, UP), np.float32)
    Rz[:U, :U] = -rec_kernel[:, :U]
    Rr = np.zeros((UP, UP), np.float32)
    Rr[:U, :U] = rec_kernel[:, U:2 * U]
    Rh = np.zeros((UP, UP), np.float32)
    Rh[:U, :U] = rec_kernel[:, 2 * U:]
    Rh[100, :U] = b1[2 * U:]      # h row 100 == 1 supplies the b1h bias
    w["wKz"], w["wKr"], w["wKh"] = Kz, Kr, Kh
    w["wRz"], w["wRr"], w["wRh"] = Rz, Rr, Rh
    w["wA1"] = A1_w
    w["wA2"] = A2_w
    w["wVr"] = np.broadcast_to(v[0][:, None], (U, U)).copy()
    return {k: val.astype(bf16) for k, val in w.items()}


def kernel(session_hidden, mask, kernel, rec_kernel, bias, A1_w, A2_w, v):
    session_hidden = np.asarray(session_hidden, np.float32)
    mask = np.asarray(mask, np.float32)
    kernel_w = np.asarray(kernel, np.float32)
    rec_kernel = np.asarray(rec_kernel, np.float32)
    bias = np.asarray(bias, np.float32)
    A1_w = np.asarray(A1_w, np.float32)
    A2_w = np.asarray(A2_w, np.float32)
    v = np.asarray(v, np.float32)

    if "nc" not in _CACHE:
        _CACHE["nc"] = _build()
    nc = _CACHE["nc"]

    w = _prep_weights(kernel_w, rec_kernel, bias, A1_w, A2_w, v)

    # xaug host layout: [NCHUNK, T//WX, 128, WX, BC]
    x = session_hidden.reshape(NCORES, NCHUNK, BC, T, E)
    m = mask.reshape(NCORES, NCHUNK, BC, T)
    in_maps = []
    for k in range(NCORES):
        xa = np.zeros((NCHUNK, T // WX, 128, WX, BC), np.float32)
        # [c, tb, e, wx, b] from x[k]: [c, b, t, e] -> transpose
        xt = x[k].transpose(0, 3, 2, 1)            # [c, e, t, b]
        xa[:, :, :E, :, :] = xt.reshape(NCHUNK, E, T // WX, WX, BC).transpose(0, 2, 1, 3, 4)
        mm = (1.0 - m[k]).transpose(0, 2, 1)        # [c, t, b]
        xa[:, :, 100, :, :] = mm.reshape(NCHUNK, T // WX, WX, BC)
        xa[:, :, 101, :, :] = 1.0
        im = dict(w)
        im["xaug"] = xa.astype(bf16)
        in_maps.append(im)

    res = bass_utils.run_bass_kernel_spmd(nc, in_maps, core_ids=list(range(NCORES)))

    out_raw = np.zeros((B, U), np.float32)
    last = np.zeros((B, U), np.float32)
    for k in range(NCORES):
        r = res.results[k]
        for c in range(NCHUNK):
            sl = slice(k * PERCORE + c * BC, k * PERCORE + (c + 1) * BC)
            out_raw[sl] = np.asarray(r["outraw"][c]).T.astype(np.float32)
            last[sl] = np.asarray(r["lastout"][c]).T.astype(np.float32)

    # host correction for masked timesteps (device used last@A1 term for ALL t)
    lengths = mask.sum(1)
    sl_ = last @ A2_w
    c_ = last @ A1_w
    sig = lambda a: 1.0 / (1.0 + np.exp(-a))
    a_corr = (sig(sl_ + c_) - sig(sl_)) @ v[0]
    out = out_raw - (T - lengths)[:, None] * a_corr[:, None] * last
    return out.astype(np.float32)


# revision 3
# speedup vs baseline: 1.2755x; 1.2593x over previous
"""Trainium2 Bass kernel for nn_LocalEncoder (masked GRU + attention pooling), v3.

kernel2 structure plus length-aware scheduling:
- Rows are globally sorted by sequence length and dealt round-robin across the
  8 cores, so every core sees an identical length profile (SPMD-safe).
- Per core: chunk0 = the 256 shortest rows (scanned only to their max length
  L0, a compile-time constant derived from the mask), chunk1 = the 256 longest
  (scanned to L1). While chunk1 finishes its solo scan, chunk0's attention
  groups are emitted interleaved to fill the dependency-chain gaps.
- Host correction accounts for masked steps the device froze (within the
  chunk's step range) or never visited (beyond it).
"""
import sys
sys.path.insert(0, "/opt/trn_rl_repo")
from contextlib import ExitStack

import numpy as np
import ml_dtypes

import concourse.bass as bass
import concourse.bacc as bacc
import concourse.tile as tile
from concourse import mybir
from concourse import bass_utils

bf16 = ml_dtypes.bfloat16
AF = mybir.ActivationFunctionType
OP = mybir.AluOpType

B, T, E, U = 4096, 200, 100, 100
NCORES = 8
BC = 256
NCHUNK = 2
PERCORE = BC * NCHUNK
WX = 8
WA = 2            # attention timestep batch (psum budget: 8 banks total)
UP = U + 1

_CACHE = {}


def _build(tc_steps):
    nc = bacc.Bacc()
    dt = mybir.dt
    L1 = max(tc_steps)
    xaug = nc.dram_tensor("xaug", [NCHUNK, L1 // WX, 128, WX, BC], dt.bfloat16,
                          kind="ExternalInput")
    wKz = nc.dram_tensor("wKz", [128, UP], dt.bfloat16, kind="ExternalInput")
    wKr = nc.dram_tensor("wKr", [128, UP], dt.bfloat16, kind="ExternalInput")
    wKh = nc.dram_tensor("wKh", [128, UP], dt.bfloat16, kind="ExternalInput")
    wRz = nc.dram_tensor("wRz", [UP, UP], dt.bfloat16, kind="ExternalInput")
    wRr = nc.dram_tensor("wRr", [UP, UP], dt.bfloat16, kind="ExternalInput")
    wRh = nc.dram_tensor("wRh", [UP, UP], dt.bfloat16, kind="ExternalInput")
    wA1 = nc.dram_tensor("wA1", [U, U], dt.bfloat16, kind="ExternalInput")
    wA2 = nc.dram_tensor("wA2", [U, U], dt.bfloat16, kind="ExternalInput")
    wVr = nc.dram_tensor("wVr", [U, U], dt.bfloat16, kind="ExternalInput")
    outraw = nc.dram_tensor("outraw", [NCHUNK, U, BC], dt.float32, kind="ExternalOutput")
    lastout = nc.dram_tensor("lastout", [NCHUNK, U, BC], dt.float32, kind="ExternalOutput")

    with tile.TileContext(nc) as tc, ExitStack() as octx:
        singles = octx.enter_context(tc.tile_pool(name="singles", bufs=1))
        dram = octx.enter_context(tc.tile_pool(name="dram", bufs=1, space="DRAM"))

        def load_w(dram_w, p, m):
            t = singles.tile([p, m], mybir.dt.bfloat16, tag=dram_w.name)
            nc.sync.dma_start(out=t, in_=dram_w[:, :])
            return t
        Kz, Kr, Kh = load_w(wKz, 128, UP), load_w(wKr, 128, UP), load_w(wKh, 128, UP)
        Rz, Rr, Rh = load_w(wRz, UP, UP), load_w(wRr, UP, UP), load_w(wRh, UP, UP)
        A1b, A2b, Vr = load_w(wA1, U, U), load_w(wA2, U, U), load_w(wVr, U, U)

        state = dram.tile([NCHUNK, U, L1, BC], mybir.dt.bfloat16)

        with ExitStack() as ctx:
            xp = ctx.enter_context(tc.tile_pool(name="xp", bufs=3))
            hp = ctx.enter_context(tc.tile_pool(name="hp", bufs=4))
            gp = ctx.enter_context(tc.tile_pool(name="gp", bufs=3))
            pzr = ctx.enter_context(tc.tile_pool(name="pzr", bufs=1, space="PSUM"))
            phc = ctx.enter_context(tc.tile_pool(name="phc", bufs=1, space="PSUM"))
            sp = ctx.enter_context(tc.tile_pool(name="sp", bufs=3))
            gp2 = ctx.enter_context(tc.tile_pool(name="gp2", bufs=3))
            ap2 = ctx.enter_context(tc.tile_pool(name="ap2", bufs=1))
            psb = ctx.enter_context(tc.tile_pool(name="psb", bufs=2, space="PSUM"))
            pal = ctx.enter_context(tc.tile_pool(name="pal", bufs=2, space="PSUM"))

            hprev = [None] * NCHUNK
            for c in range(NCHUNK):
                h0 = hp.tile([128, BC], mybir.dt.bfloat16, tag=f"h{c}")
                nc.vector.memset(h0, 0.0)
                nc.vector.memset(h0[96:128, :], 1.0)
                nc.vector.memset(h0[96:100, :], 0.0)
                hprev[c] = h0

            last_tiles = [None] * NCHUNK
            c4s = [None] * NCHUNK
            accs = [None] * NCHUNK
            attn_done = [0] * NCHUNK

            def finish_scan(c):
                lt = singles.tile([128, BC], mybir.dt.bfloat16, tag=f"last{c}",
                                  name=f"last{c}")
                nc.vector.tensor_copy(lt[0:UP, :], hprev[c][0:UP, :])
                last_tiles[c] = lt
                lo = singles.tile([U, BC], mybir.dt.float32, tag=f"lasto{c}",
                                  name=f"lasto{c}")
                nc.vector.tensor_copy(lo, hprev[c][0:U, :])
                nc.scalar.dma_start(out=lastout[c, :, :], in_=lo)
                sb1 = psb.tile([128, WA, BC], mybir.dt.float32, tag="sb", name="sb1")
                nc.tensor.matmul(sb1[0:U, 0, :], lhsT=A1b, rhs=lt[0:U, :],
                                 start=True, stop=True)
                c4 = ap2.tile([U, WA, BC], mybir.dt.bfloat16, tag=f"c4{c}",
                              name=f"c4{c}")
                nc.vector.tensor_copy(c4[:, 0, :], sb1[0:U, 0, :])
                nc.gpsimd.tensor_copy(c4[:, 1, :], c4[:, 0, :])
                c4s[c] = c4
                acc = ap2.tile([U, WA, BC], mybir.dt.float32, tag=f"acc{c}",
                               name=f"acc{c}")
                nc.vector.memset(acc, 0.0)
                accs[c] = acc

            def attn_group(c, g):
                st4 = sp.tile([U, WA, BC], mybir.dt.bfloat16, tag="st", name="st4")
                nc.sync.dma_start(out=st4,
                                  in_=state[c, :, g * WA:(g + 1) * WA, :])
                sb4 = psb.tile([128, WA, BC], mybir.dt.float32, tag="sb", name="sb4")
                nc.tensor.matmul(sb4[0:U, :, :], lhsT=A2b, rhs=st4[:, :, :],
                                 start=True, stop=True)
                sbc = gp2.tile([U, WA, BC], mybir.dt.bfloat16, tag="sbc", name="sbc")
                nc.vector.tensor_tensor(sbc, sb4[0:U, :, :], c4s[c], OP.add)
                g4 = gp2.tile([U, WA, BC], mybir.dt.bfloat16, tag="g4", name="g4")
                nc.scalar.activation(g4, sbc, AF.Sigmoid)
                al4 = pal.tile([128, WA, BC], mybir.dt.float32, tag="al", name="al4")
                nc.tensor.matmul(al4[0:U, :, :], lhsT=Vr, rhs=g4[:, :, :],
                                 start=True, stop=True)
                tmp = gp2.tile([U, WA, BC], mybir.dt.bfloat16, tag="tmp", name="tmp")
                nc.vector.tensor_tensor(tmp, al4[0:U, :, :], st4, OP.mult)
                nc.gpsimd.tensor_tensor(accs[c], accs[c], tmp, OP.add)
                attn_done[c] += 1

            xts = [None] * NCHUNK
            for t in range(L1):
                alive = [c for c in range(NCHUNK) if t < tc_steps[c]]
                ib = t % WX
                if ib == 0:
                    for c in alive:
                        xt = xp.tile([128, WX, BC], mybir.dt.bfloat16, tag=f"x{c}")
                        nc.sync.dma_start(out=xt, in_=xaug[c, t // WX, :, :, :])
                        xts[c] = xt
                zr = [None] * NCHUNK
                hc = [None] * NCHUNK
                for c in alive:
                    zr[c] = pzr.tile([128, 2, BC], mybir.dt.float32, tag=f"zr{c}",
                                     name=f"zr{c}")
                    hc[c] = phc.tile([128, 2, BC], mybir.dt.float32, tag=f"hc{c}",
                                     name=f"hc{c}")
                for c in alive:
                    nc.tensor.matmul(zr[c][0:UP, 0, :], lhsT=Kz, rhs=xts[c][:, ib, :],
                                     start=True, stop=False, skip_group_check=True)
                for c in alive:
                    nc.tensor.matmul(zr[c][0:UP, 0, :], lhsT=Rz, rhs=hprev[c][0:UP, :],
                                     start=False, stop=True, skip_group_check=True)
                for c in alive:
                    nc.tensor.matmul(zr[c][0:UP, 1, :], lhsT=Kr, rhs=xts[c][:, ib, :],
                                     start=True, stop=False, skip_group_check=True)
                for c in alive:
                    nc.tensor.matmul(zr[c][0:UP, 1, :], lhsT=Rr, rhs=hprev[c][0:UP, :],
                                     start=False, stop=True, skip_group_check=True)
                for c in alive:
                    nc.tensor.matmul(hc[c][0:UP, 0, :], lhsT=Kh, rhs=xts[c][:, ib, :],
                                     start=True, stop=True, skip_group_check=True)
                for c in alive:
                    nc.tensor.matmul(hc[c][0:UP, 1, :], lhsT=Rh, rhs=hprev[c][0:UP, :],
                                     start=True, stop=True, skip_group_check=True)
                for c in alive:
                    h = hprev[c]
                    zrs = gp.tile([UP, 2, BC], mybir.dt.bfloat16, tag=f"zrs{c}")
                    nc.scalar.activation(zrs, zr[c][0:UP, :, :], AF.Sigmoid)
                    t1 = gp.tile([UP, BC], mybir.dt.bfloat16, tag=f"t1{c}")
                    nc.vector.tensor_tensor(t1, zrs[:, 1, :], hc[c][0:UP, 1, :], OP.mult)
                    s = gp.tile([UP, BC], mybir.dt.bfloat16, tag=f"s{c}")
                    nc.vector.tensor_tensor(s, hc[c][0:UP, 0, :], t1, OP.add)
                    hh = gp.tile([UP, BC], mybir.dt.bfloat16, tag=f"hh{c}")
                    nc.scalar.activation(hh, s, AF.Tanh)
                    d = gp.tile([UP, BC], mybir.dt.bfloat16, tag=f"d{c}")
                    nc.vector.tensor_tensor(d, hh, h[0:UP, :], OP.subtract)
                    e = gp.tile([UP, BC], mybir.dt.bfloat16, tag=f"e{c}")
                    nc.vector.tensor_tensor(e, zrs[:, 0, :], d, OP.mult)
                    hn = hp.tile([128, BC], mybir.dt.bfloat16, tag=f"h{c}")
                    nc.vector.tensor_tensor(hn[0:UP, :], e, h[0:UP, :], OP.add)
                    nc.scalar.dma_start(out=state[c, :, t, :], in_=hn[0:U, :])
                    hprev[c] = hn
                for c in range(NCHUNK):
                    if t + 1 == tc_steps[c] and last_tiles[c] is None:
                        finish_scan(c)
                for c in range(NCHUNK):
                    if last_tiles[c] is not None and attn_done[c] < tc_steps[c] // WA:
                        attn_group(c, attn_done[c])
                        break

            for c in range(NCHUNK):
                while attn_done[c] < tc_steps[c] // WA:
                    attn_group(c, attn_done[c])

            for c in range(NCHUNK):
                osb = gp2.tile([U, BC], mybir.dt.float32, tag=f"osb{c}")
                nc.vector.tensor_reduce(
                    osb, accs[c].rearrange("u w b -> u b w"), mybir.AxisListType.X,
                    OP.add)
                nc.scalar.dma_start(out=outraw[c, :, :], in_=osb)

    nc.compile()
    return nc


def _prep_weights(kernel_w, rec_kernel, bias, A1_w, A2_w, v):
    b0, b1 = bias[0], bias[1]
    w = {}
    Kz = np.zeros((128, UP), np.float32)
    Kz[:E, :U] = -kernel_w[:, :U]
    Kz[100, :U] = -40.0
    Kz[101, :U] = -(b0[:U] + b1[:U])
    Kz[101, 100] = -40.0
    Kr = np.zeros((128, UP), np.float32)
    Kr[:E, :U] = kernel_w[:, U:2 * U]
    Kr[101, :U] = b0[U:2 * U] + b1[U:2 * U]
    Kh = np.zeros((128, UP), np.float32)
    Kh[:E, :U] = kernel_w[:, 2 * U:]
    Kh[101, :U] = b0[2 * U:]
    Rz = np.zeros((UP, UP), np.float32)
    Rz[:U, :U] = -rec_kernel[:, :U]
    Rr = np.zeros((UP, UP), np.float32)
    Rr[:U, :U] = rec_kernel[:, U:2 * U]
    Rh = np.zeros((UP, UP), np.float32)
    Rh[:U, :U] = rec_kernel[:, 2 * U:]
    Rh[100, :U] = b1[2 * U:]
    w["wKz"], w["wKr"], w["wKh"] = Kz, Kr, Kh
    w["wRz"], w["wRr"], w["wRh"] = Rz, Rr, Rh
    w["wA1"] = A1_w
    w["wA2"] = A2_w
    w["wVr"] = np.broadcast_to(v[0][:, None], (U, U)).copy()
    return {k: val.astype(bf16) for k, val in w.items()}


def _schedule(mask):
    lengths = mask.sum(1).astype(np.int64)
    order = np.argsort(lengths, kind="stable")       # ascending
    Ls = lengths[order]
    L0 = int(Ls[BC * NCORES - 1])                    # max len among chunk-0 rows
    L1 = int(Ls[-1])
    rup = lambda a: min(T, ((a + WX - 1) // WX) * WX)
    return order, [max(WX, rup(L0)), max(WX, rup(L1))]


def _make_inmaps(session_hidden, mask, w, order, tc_steps):
    L1 = max(tc_steps)
    xs = session_hidden[order]
    ms = mask[order]
    xs = xs.reshape(PERCORE, NCORES, T, E)           # [slot, core, t, e]
    ms = ms.reshape(PERCORE, NCORES, T)
    in_maps = []
    for k in range(NCORES):
        xk = xs[:, k].reshape(NCHUNK, BC, T, E)
        mk = ms[:, k].reshape(NCHUNK, BC, T)
        xa = np.zeros((NCHUNK, L1 // WX, 128, WX, BC), np.float32)
        xt = xk.transpose(0, 3, 2, 1)[:, :, :L1, :]  # [c, e, t, j]
        xa[:, :, :E, :, :] = xt.reshape(NCHUNK, E, L1 // WX, WX, BC).transpose(0, 2, 1, 3, 4)
        mm = (1.0 - mk).transpose(0, 2, 1)[:, :L1, :]
        xa[:, :, 100, :, :] = mm.reshape(NCHUNK, L1 // WX, WX, BC)
        xa[:, :, 101, :, :] = 1.0
        im = dict(w)
        im["xaug"] = xa.astype(bf16)
        in_maps.append(im)
    return in_maps


def kernel(session_hidden, mask, kernel, rec_kernel, bias, A1_w, A2_w, v):
    session_hidden = np.asarray(session_hidden, np.float32)
    mask = np.asarray(mask, np.float32)
    kernel_w = np.asarray(kernel, np.float32)
    rec_kernel = np.asarray(rec_kernel, np.float32)
    bias = np.asarray(bias, np.float32)
    A1_w = np.asarray(A1_w, np.float32)
    A2_w = np.asarray(A2_w, np.float32)
    v = np.asarray(v, np.float32)

    order, tc_steps = _schedule(mask)
    key = tuple(tc_steps)
    if _CACHE.get("key") != key:
        _CACHE["nc"] = _build(tc_steps)
        _CACHE["key"] = key
    nc = _CACHE["nc"]

    w = _prep_weights(kernel_w, rec_kernel, bias, A1_w, A2_w, v)
    in_maps = _make_inmaps(session_hidden, mask, w, order, tc_steps)
    res = bass_utils.run_bass_kernel_spmd(nc, in_maps, core_ids=list(range(NCORES)))

    out_s = np.zeros((B, U), np.float32)
    last_s = np.zeros((B, U), np.float32)
    tcs_row = np.zeros(B, np.float32)
    for k in range(NCORES):
        r = res.results[k]
        for c in range(NCHUNK):
            ranks = (np.arange(BC) + c * BC) * NCORES + k
            out_s[ranks] = np.asarray(r["outraw"][c]).T.astype(np.float32)
            last_s[ranks] = np.asarray(r["lastout"][c]).T.astype(np.float32)
            tcs_row[ranks] = tc_steps[c]

    lengths_s = mask.sum(1)[order]
    sl_ = last_s @ A2_w
    c_ = last_s @ A1_w
    sig = lambda a: 1.0 / (1.0 + np.exp(-a))
    a_dev = sig(sl_ + c_) @ v[0]
    a_true = sig(sl_) @ v[0]
    n_dev = tcs_row - lengths_s
    out_sorted = (out_s
                  - n_dev[:, None] * (a_dev - a_true)[:, None] * last_s
                  + (T - tcs_row)[:, None] * a_true[:, None] * last_s)
    out = np.zeros((B, U), np.float32)
    out[order] = out_sorted
    return out.astype(np.float32)


# revision 4
# speedup vs baseline: 1.3439x; 1.0536x over previous
"""Trainium2 Bass kernel for nn_LocalEncoder, v4.

v3 plus time-splitting of the long chunk: chunk1 (the 256 longest rows per
core) is scanned by TWO concurrent chains — chain1 covers t in [0, Ls), chain2
covers t in [Ls, T) after a 16-step warm-up from h=0 (the GRU's update gate
contracts ~0.5x/step, so the initial state is forgotten to ~1e-5 within the
warm-up). Rows with len <= Ls are fed an all-masked mask in chain2, so their
chain2 state stays exactly 0 and contributes nothing; their true last state
comes from chain1 via an on-device select. Three near-equal chains scan
concurrently (no solo tail); attention runs as a post-scan W=4 pipelined phase.
Host correction uses a per-row device step count.
"""
import sys
sys.path.insert(0, "/opt/trn_rl_repo")
from contextlib import ExitStack

import numpy as np
import ml_dtypes

import concourse.bass as bass
import concourse.bacc as bacc
import concourse.tile as tile
from concourse import mybir
from concourse import bass_utils

bf16 = ml_dtypes.bfloat16
AF = mybir.ActivationFunctionType
OP = mybir.AluOpType

B, T, E, U = 4096, 200, 100, 100
NCORES = 8
BC = 256
NCHUNK = 2
PERCORE = BC * NCHUNK
WX = 8
WA = 4
UP = U + 1

_CACHE = {}


def _plan(L0, L1):
    """Chain plan: (chunk, t0, t1, warm). Chain2 warm-starts WU before Ls."""
    WU = 16 if L1 >= 64 else 8
    rup = lambda a: ((a + WX - 1) // WX) * WX
    Ls = min(L1, rup((L1 + WU) // 2))
    if Ls >= L1:           # degenerate: no split
        return [(0, 0, L0, 0), (1, 0, L1, 0)], L1
    return [(0, 0, L0, 0), (1, 0, Ls, 0), (1, Ls - WU, L1, WU)], Ls


def _build(L0, L1):
    chains, Ls = _plan(L0, L1)
    NCH = len(chains)
    nblk = max((t1 - t0) // WX for _, t0, t1, _ in chains)
    nc = bacc.Bacc()
    dt = mybir.dt
    xaug = nc.dram_tensor("xaug", [NCH, nblk, 128, WX, BC], dt.bfloat16,
                          kind="ExternalInput")
    wmsel = nc.dram_tensor("wmsel", [128, BC], dt.bfloat16, kind="ExternalInput")
    wKz = nc.dram_tensor("wKz", [128, UP], dt.bfloat16, kind="ExternalInput")
    wKr = nc.dram_tensor("wKr", [128, UP], dt.bfloat16, kind="ExternalInput")
    wKh = nc.dram_tensor("wKh", [128, UP], dt.bfloat16, kind="ExternalInput")
    wRz = nc.dram_tensor("wRz", [UP, UP], dt.bfloat16, kind="ExternalInput")
    wRr = nc.dram_tensor("wRr", [UP, UP], dt.bfloat16, kind="ExternalInput")
    wRh = nc.dram_tensor("wRh", [UP, UP], dt.bfloat16, kind="ExternalInput")
    wA1 = nc.dram_tensor("wA1", [U, U], dt.bfloat16, kind="ExternalInput")
    wA2 = nc.dram_tensor("wA2", [U, U], dt.bfloat16, kind="ExternalInput")
    wVr = nc.dram_tensor("wVr", [U, U], dt.bfloat16, kind="ExternalInput")
    outraw = nc.dram_tensor("outraw", [NCHUNK, U, BC], dt.float32, kind="ExternalOutput")
    lastout = nc.dram_tensor("lastout", [NCHUNK, U, BC], dt.float32, kind="ExternalOutput")

    with tile.TileContext(nc) as tc, ExitStack() as octx:
        singles = octx.enter_context(tc.tile_pool(name="singles", bufs=1))
        dram = octx.enter_context(tc.tile_pool(name="dram", bufs=1, space="DRAM"))

        def load_w(dram_w, p, m):
            t = singles.tile([p, m], mybir.dt.bfloat16, tag=dram_w.name)
            nc.sync.dma_start(out=t, in_=dram_w[:, :])
            return t
        Kz, Kr, Kh = load_w(wKz, 128, UP), load_w(wKr, 128, UP), load_w(wKh, 128, UP)
        Rz, Rr, Rh = load_w(wRz, UP, UP), load_w(wRr, UP, UP), load_w(wRh, UP, UP)
        A1b, A2b, Vr = load_w(wA1, U, U), load_w(wA2, U, U), load_w(wVr, U, U)
        msel = load_w(wmsel, 128, BC)

        state = dram.tile([NCHUNK, U, L1, BC], mybir.dt.bfloat16)
        hfin = [None] * NCH

        # ---------------- scan: all chains concurrently ----------------
        with ExitStack() as ctx:
            xp = ctx.enter_context(tc.tile_pool(name="xp", bufs=3))
            hp = ctx.enter_context(tc.tile_pool(name="hp", bufs=4))
            gp = ctx.enter_context(tc.tile_pool(name="gp", bufs=3))
            pzr = ctx.enter_context(tc.tile_pool(name="pzr", bufs=1, space="PSUM"))
            phc = ctx.enter_context(tc.tile_pool(name="phc", bufs=1, space="PSUM"))

            hprev = [None] * NCH
            for i in range(NCH):
                h0 = hp.tile([128, BC], mybir.dt.bfloat16, tag=f"h{i}", name=f"h{i}")
                nc.vector.memset(h0, 0.0)
                nc.vector.memset(h0[96:128, :], 1.0)
                nc.vector.memset(h0[96:100, :], 0.0)
                hprev[i] = h0

            xts = [None] * NCH
            rounds = max(t1 - t0 for _, t0, t1, _ in chains)
            for r in range(rounds):
                alive = [i for i, (_, t0, t1, _) in enumerate(chains)
                         if r < t1 - t0]
                ib = r % WX
                if ib == 0:
                    for i in alive:
                        xt = xp.tile([128, WX, BC], mybir.dt.bfloat16, tag=f"x{i}",
                                     name=f"x{i}")
                        nc.sync.dma_start(out=xt, in_=xaug[i, r // WX, :, :, :])
                        xts[i] = xt
                zr = [None] * NCH
                hc = [None] * NCH
                for i in alive:
                    zr[i] = pzr.tile([128, 2, BC], mybir.dt.float32, tag=f"zr{i}",
                                     name=f"zr{i}")
                    hc[i] = phc.tile([128, 2, BC], mybir.dt.float32, tag=f"hc{i}",
                                     name=f"hc{i}")
                for i in alive:
                    nc.tensor.matmul(zr[i][0:UP, 0, :], lhsT=Kz, rhs=xts[i][:, ib, :],
                                     start=True, stop=False, skip_group_check=True)
                for i in alive:
                    nc.tensor.matmul(zr[i][0:UP, 0, :], lhsT=Rz, rhs=hprev[i][0:UP, :],
                                     start=False, stop=True, skip_group_check=True)
                for i in alive:
                    nc.tensor.matmul(zr[i][0:UP, 1, :], lhsT=Kr, rhs=xts[i][:, ib, :],
                                     start=True, stop=False, skip_group_check=True)
                for i in alive:
                    nc.tensor.matmul(zr[i][0:UP, 1, :], lhsT=Rr, rhs=hprev[i][0:UP, :],
                                     start=False, stop=True, skip_group_check=True)
                for i in alive:
                    nc.tensor.matmul(hc[i][0:UP, 0, :], lhsT=Kh, rhs=xts[i][:, ib, :],
                                     start=True, stop=True, skip_group_check=True)
                for i in alive:
                    nc.tensor.matmul(hc[i][0:UP, 1, :], lhsT=Rh, rhs=hprev[i][0:UP, :],
                                     start=True, stop=True, skip_group_check=True)
                for i in alive:
                    ch, t0, t1, warm = chains[i]
                    t = t0 + r
                    h = hprev[i]
                    zrs = gp.tile([UP, 2, BC], mybir.dt.bfloat16, tag=f"zrs{i}",
                                  name=f"zrs{i}")
                    nc.scalar.activation(zrs, zr[i][0:UP, :, :], AF.Sigmoid)
                    t1t = gp.tile([UP, BC], mybir.dt.bfloat16, tag=f"t1{i}",
                                  name=f"t1{i}")
                    nc.vector.tensor_tensor(t1t, zrs[:, 1, :], hc[i][0:UP, 1, :],
                                            OP.mult)
                    s = gp.tile([UP, BC], mybir.dt.bfloat16, tag=f"s{i}", name=f"s{i}")
                    nc.vector.tensor_tensor(s, hc[i][0:UP, 0, :], t1t, OP.add)
                    hh = gp.tile([UP, BC], mybir.dt.bfloat16, tag=f"hh{i}",
                                 name=f"hh{i}")
                    nc.scalar.activation(hh, s, AF.Tanh)
                    d = gp.tile([UP, BC], mybir.dt.bfloat16, tag=f"d{i}", name=f"d{i}")
                    nc.vector.tensor_tensor(d, hh, h[0:UP, :], OP.subtract)
                    e = gp.tile([UP, BC], mybir.dt.bfloat16, tag=f"e{i}", name=f"e{i}")
                    nc.vector.tensor_tensor(e, zrs[:, 0, :], d, OP.mult)
                    hn = hp.tile([128, BC], mybir.dt.bfloat16, tag=f"h{i}",
                                 name=f"hn{i}")
                    nc.vector.tensor_tensor(hn[0:UP, :], e, h[0:UP, :], OP.add)
                    if r >= warm:
                        nc.scalar.dma_start(out=state[ch, :, t, :], in_=hn[0:U, :])
                    hprev[i] = hn

            for i in range(NCH):
                hf = singles.tile([128, BC], mybir.dt.bfloat16, tag=f"hf{i}",
                                  name=f"hf{i}")
                nc.vector.tensor_copy(hf[0:UP, :], hprev[i][0:UP, :])
                hfin[i] = hf

        # ---------------- attention ----------------
        NG = [L0 // WA, L1 // WA]
        with ExitStack() as ctx:
            sp = ctx.enter_context(tc.tile_pool(name="sp", bufs=3))
            gp2 = ctx.enter_context(tc.tile_pool(name="gp2", bufs=3))
            ap2 = ctx.enter_context(tc.tile_pool(name="ap2", bufs=1))
            psb = ctx.enter_context(tc.tile_pool(name="psb", bufs=2, space="PSUM"))
            pal = ctx.enter_context(tc.tile_pool(name="pal", bufs=2, space="PSUM"))

            lasts, c4s, accs = [], [], []
            for c in range(NCHUNK):
                lt = ap2.tile([128, BC], mybir.dt.bfloat16, tag=f"lt{c}",
                              name=f"lt{c}")
                if c == 0 or len(chains) == 2:
                    nc.vector.tensor_copy(lt[0:UP, :], hfin[c][0:UP, :])
                else:
                    # lt = h1 + msel * (h2 - h1)
                    df = gp2.tile([UP, BC], mybir.dt.bfloat16, tag="df", name="df")
                    nc.vector.tensor_tensor(df, hfin[2][0:UP, :], hfin[1][0:UP, :],
                                            OP.subtract)
                    nc.vector.tensor_tensor(df, df, msel[0:UP, :], OP.mult)
                    nc.vector.tensor_tensor(lt[0:UP, :], hfin[1][0:UP, :], df,
                                            OP.add)
                lasts.append(lt)
                lo = gp2.tile([U, BC], mybir.dt.float32, tag="lo", name="lo")
                nc.vector.tensor_copy(lo, lt[0:U, :])
                nc.scalar.dma_start(out=lastout[c, :, :], in_=lo)
                sb1 = psb.tile([128, WA, BC], mybir.dt.float32, tag="sb", name="sb1")
                nc.tensor.matmul(sb1[0:U, 0, :], lhsT=A1b, rhs=lt[0:U, :],
                                 start=True, stop=True)
                c4 = ap2.tile([U, WA, BC], mybir.dt.bfloat16, tag=f"c4{c}",
                              name=f"c4{c}")
                nc.vector.tensor_copy(c4[:, 0, :], sb1[0:U, 0, :])
                nc.gpsimd.tensor_copy(c4[:, 1, :], c4[:, 0, :])
                nc.gpsimd.tensor_copy(c4[:, 2:4, :], c4[:, 0:2, :])
                c4s.append(c4)
                acc = ap2.tile([U, WA, BC], mybir.dt.float32, tag=f"acc{c}",
                               name=f"acc{c}")
                nc.vector.memset(acc, 0.0)
                accs.append(acc)

            order = [(c, g) for g in range(max(NG)) for c in range(NCHUNK)
                     if g < NG[c]]
            for c, g in order:
                st4 = sp.tile([U, WA, BC], mybir.dt.bfloat16, tag="st", name="st4")
                nc.sync.dma_start(out=st4, in_=state[c, :, g * WA:(g + 1) * WA, :])
                sb4 = psb.tile([128, WA, BC], mybir.dt.float32, tag="sb", name="sb4")
                nc.tensor.matmul(sb4[0:U, 0:2, :], lhsT=A2b, rhs=st4[:, 0:2, :],
                                 start=True, stop=True)
                nc.tensor.matmul(sb4[0:U, 2:4, :], lhsT=A2b, rhs=st4[:, 2:4, :],
                                 start=True, stop=True)
                sbc = gp2.tile([U, WA, BC], mybir.dt.bfloat16, tag="sbc", name="sbc")
                nc.vector.tensor_tensor(sbc, sb4[0:U, :, :], c4s[c], OP.add)
                g4 = gp2.tile([U, WA, BC], mybir.dt.bfloat16, tag="g4", name="g4")
                nc.scalar.activation(g4, sbc, AF.Sigmoid)
                al4 = pal.tile([128, WA, BC], mybir.dt.float32, tag="al", name="al4")
                nc.tensor.matmul(al4[0:U, 0:2, :], lhsT=Vr, rhs=g4[:, 0:2, :],
                                 start=True, stop=True)
                nc.tensor.matmul(al4[0:U, 2:4, :], lhsT=Vr, rhs=g4[:, 2:4, :],
                                 start=True, stop=True)
                tmp = gp2.tile([U, WA, BC], mybir.dt.bfloat16, tag="tmp", name="tmp")
                nc.vector.tensor_tensor(tmp, al4[0:U, :, :], st4, OP.mult)
                if c == 0:
                    nc.gpsimd.tensor_tensor(accs[c], accs[c], tmp, OP.add)
                else:
                    nc.vector.tensor_tensor(accs[c], accs[c], tmp, OP.add)

            for c in range(NCHUNK):
                osb = gp2.tile([U, BC], mybir.dt.float32, tag=f"osb{c}",
                               name=f"osb{c}")
                nc.vector.tensor_reduce(
                    osb, accs[c].rearrange("u w b -> u b w"), mybir.AxisListType.X,
                    OP.add)
                nc.scalar.dma_start(out=outraw[c, :, :], in_=osb)

    nc.compile()
    return nc, chains, Ls


def _prep_weights(kernel_w, rec_kernel, bias, A1_w, A2_w, v):
    b0, b1 = bias[0], bias[1]
    w = {}
    Kz = np.zeros((128, UP), np.float32)
    Kz[:E, :U] = -kernel_w[:, :U]
    Kz[100, :U] = -40.0
    Kz[101, :U] = -(b0[:U] + b1[:U])
    Kz[101, 100] = -40.0
    Kr = np.zeros((128, UP), np.float32)
    Kr[:E, :U] = kernel_w[:, U:2 * U]
    Kr[101, :U] = b0[U:2 * U] + b1[U:2 * U]
    Kh = np.zeros((128, UP), np.float32)
    Kh[:E, :U] = kernel_w[:, 2 * U:]
    Kh[101, :U] = b0[2 * U:]
    Rz = np.zeros((UP, UP), np.float32)
    Rz[:U, :U] = -rec_kernel[:, :U]
    Rr = np.zeros((UP, UP), np.float32)
    Rr[:U, :U] = rec_kernel[:, U:2 * U]
    Rh = np.zeros((UP, UP), np.float32)
    Rh[:U, :U] = rec_kernel[:, 2 * U:]
    Rh[100, :U] = b1[2 * U:]
    w["wKz"], w["wKr"], w["wKh"] = Kz, Kr, Kh
    w["wRz"], w["wRr"], w["wRh"] = Rz, Rr, Rh
    w["wA1"] = A1_w
    w["wA2"] = A2_w
    w["wVr"] = np.broadcast_to(v[0][:, None], (U, U)).copy()
    return {k: val.astype(bf16) for k, val in w.items()}


def _schedule(mask):
    lengths = mask.sum(1).astype(np.int64)
    order = np.argsort(lengths, kind="stable")
    Ls_ = lengths[order]
    L0 = int(Ls_[BC * NCORES - 1])
    L1 = int(Ls_[-1])
    rup = lambda a: min(T, ((a + WX - 1) // WX) * WX)
    return order, max(WX, rup(L0)), max(WX, rup(L1))


def _make_inmaps(session_hidden, mask, w, order, L0, L1, chains):
    nblk = max((t1 - t0) // WX for _, t0, t1, _ in chains)
    xs = session_hidden[order].reshape(PERCORE, NCORES, T, E)
    ms = mask[order].reshape(PERCORE, NCORES, T)
    Ls = chains[1][2] if len(chains) > 2 else L1
    in_maps = []
    for k in range(NCORES):
        xk = xs[:, k].reshape(NCHUNK, BC, T, E)
        mk = ms[:, k].reshape(NCHUNK, BC, T)
        lens_k = mk.sum(2)                       # [c, j]
        xa = np.zeros((len(chains), nblk, 128, WX, BC), np.float32)
        for i, (c, t0, t1, warm) in enumerate(chains):
            nb = (t1 - t0) // WX
            xseg = xk[c, :, t0:t1, :].transpose(2, 1, 0)    # [e, t, j] -> wait
            # xk[c] is [j, t, e]; want [e, tseg, j]
            xseg = xk[c, :, t0:t1, :].transpose(2, 1, 0)    # [e, tseg, j]
            xa[i, :nb, :E] = xseg.reshape(E, nb, WX, BC).transpose(1, 0, 2, 3)
            mseg = 1.0 - mk[c, :, t0:t1].T                  # [tseg, j]
            if warm > 0:
                # rows fully handled by the earlier chain: force all-masked
                dead = lens_k[c] <= Ls
                mseg = mseg.copy()
                mseg[:, dead] = 1.0
            xa[i, :nb, 100] = mseg.reshape(nb, WX, BC)
            xa[i, :nb, 101] = 1.0
        im = dict(w)
        im["xaug"] = xa.astype(bf16)
        msel = np.zeros((128, BC), np.float32)
        msel[:, :] = (lens_k[1] > Ls).astype(np.float32)[None, :]
        im["wmsel"] = msel.astype(bf16)
        in_maps.append(im)
    return in_maps


def kernel(session_hidden, mask, kernel, rec_kernel, bias, A1_w, A2_w, v):
    session_hidden = np.asarray(session_hidden, np.float32)
    mask = np.asarray(mask, np.float32)
    kernel_w = np.asarray(kernel, np.float32)
    rec_kernel = np.asarray(rec_kernel, np.float32)
    bias = np.asarray(bias, np.float32)
    A1_w = np.asarray(A1_w, np.float32)
    A2_w = np.asarray(A2_w, np.float32)
    v = np.asarray(v, np.float32)

    order, L0, L1 = _schedule(mask)
    key = (L0, L1)
    if _CACHE.get("key") != key:
        _CACHE["nc"], _CACHE["chains"], _CACHE["Ls"] = _build(L0, L1)
        _CACHE["key"] = key
    nc, chains, Ls = _CACHE["nc"], _CACHE["chains"], _CACHE["Ls"]

    w = _prep_weights(kernel_w, rec_kernel, bias, A1_w, A2_w, v)
    in_maps = _make_inmaps(session_hidden, mask, w, order, L0, L1, chains)
    res = bass_utils.run_bass_kernel_spmd(nc, in_maps, core_ids=list(range(NCORES)))

    out_s = np.zeros((B, U), np.float32)
    last_s = np.zeros((B, U), np.float32)
    tcs_row = np.zeros(B, np.float32)
    lengths_s = mask.sum(1)[order]
    for k in range(NCORES):
        r = res.results[k]
        for c in range(NCHUNK):
            ranks = (np.arange(BC) + c * BC) * NCORES + k
            out_s[ranks] = np.asarray(r["outraw"][c]).T.astype(np.float32)
            last_s[ranks] = np.asarray(r["lastout"][c]).T.astype(np.float32)
            if c == 0:
                tcs_row[ranks] = L0
            elif len(chains) > 2:
                ln = lengths_s[ranks]
                tcs_row[ranks] = np.where(ln <= Ls, float(Ls), float(L1))
            else:
                tcs_row[ranks] = L1

    sl_ = last_s @ A2_w
    c_ = last_s @ A1_w
    sig = lambda a: 1.0 / (1.0 + np.exp(-a))
    a_dev = sig(sl_ + c_) @ v[0]
    a_true = sig(sl_) @ v[0]
    n_dev = tcs_row - lengths_s
    out_sorted = (out_s
                  - n_dev[:, None] * (a_dev - a_true)[:, None] * last_s
                  + (T - tcs_row)[:, None] * a_true[:, None] * last_s)
    out = np.zeros((B, U), np.float32)
    out[order] = out_sorted
    return out.astype(np.float32)
